# revision 55
# baseline (speedup 1.0000x reference)
"""Trainium2 Bass kernel for nn_ArielDecoderCell1 (arithmetic-decoding LSTM cell).

Math summary (for the harness inputs: timeStep=20 > 0, tokens all PAD=0):
  - initial_softmax = LSTM-LM(PAD) = softmax(0) = uniform 1/V exactly
    (PAD is masked, h stays 0 — independent of the weights).
  - timeStep > 0  =>  unfolding_point := input_point, one_softmax := uniform.
  - decode on the uniform grid: cums k = (k+1)/4096 exactly in f32, so
    token = floor(p*4096) and new_coord = frac(p*4096), both bit-exact.
  - second LSTM over tokens[:, :timeStep+1]: steps 0..timeStep-1 are PAD
    (masked, h=c stay 0), so only ONE step runs from h=c=0:
        z = E[token] @ Wi + b      (h @ Wh == 0 exactly -> Wh unused,
                                    f-gate * c == 0 exactly -> f-gate unused)
        c = sigmoid(z_i) * tanh(z_g);  h = sigmoid(z_o) * tanh(c)
        one_softmax = softmax(h)   (rows with token==0 keep h=0 -> uniform)

Device strategy (8 cores): tensor-parallel over the hidden dim V=4096.
Core k owns hidden slice [k*512,(k+1)*512) and loads only the i/g/o gate
columns of Wi for its slice ([256,1536] = 1.5MB vs 64MB naive), computes its
h-shard + exp + local row-sums, AllGathers the 8 [128,1] half-sum vectors
(512B each), normalizes and writes its 512-column shard of the softmax.
The decode (token/new_coord) is computed redundantly on every core (a few
[64,1] vector ops).

Perf notes (instruction-cost model ~15.8us/core + ~5us AllGather):
- gate matmuls run as float32r (1 cycle/row vs 4 for fp32; measured
  one_softmax rel err on HW 1.3e-6 vs 3.6e-7 full-fp32)
- sigmoid(z) = 0.5*tanh(0.5 z)+0.5 so the whole cell + exp uses the single
  `exp_and_others` ACT table set (one prefetched ~2.7us table load)
- dummy bf16 matmuls from t=0 hold the PE HAM clock gate at full rate
- ACT queue is ordered to never head-of-line block on DVE products: i/g
  tanhs run full-width (fewer per-op overheads), then the o-gate tanh,
  tanh(c) and exp run as 256-column halves so the half-0 tail (o_t, h,
  exp, collective-input DMA) pipelines against half 1
- exp uses a per-partition `scale` AP to apply the PAD mask and `accum_out`
  to fuse the softmax row-sum; the final normalize+store is split across
  the two HWDGE queues so the first store overlaps the second multiply
"""

import numpy as np

B, V, EMB, LAT, MAXLEN = 64, 4096, 256, 16, 64
PAD = 0
NCORES = 8
SH = V // NCORES  # 512 hidden units per core

# "mod": token = floor(x) via int-cast + compare fixup (fast)
# "count": token = sum_k 1[k <= x] over an iota grid (slower, bulletproof)
TOKEN_MODE = "mod"
F32R = True  # run gate matmuls in float32r PE mode (4x faster, ~bf16x2 precision)
TRACE = False  # test.py can set kernel.TRACE=True to capture an NTFF profile
LAST_RESULTS = None  # BassKernelResults of the last device run (for test.py)

_prog_cache: dict = {}
_runner_cache: dict = {}


# ----------------------------------------------------------------------------
# Device program
# ----------------------------------------------------------------------------
def _build_program(cd: int, ts: int, use_bias: bool, token_mode: str,
                   no_cc: bool = False, f32r: bool = F32R):
    from concourse import bacc, bass, mybir, tile
    from concourse.masks import make_identity

    f32 = mybir.dt.float32
    i32 = mybir.dt.int32
    AF = mybir.ActivationFunctionType
    OP = mybir.AluOpType

    nc = bacc.Bacc("TRN2", target_bir_lowering=False, debug=False,
                   num_devices=NCORES)
    f32m = mybir.dt.float32r if f32r else f32  # matmul operand dtype

    ip_d = nc.dram_tensor("ip", [B, LAT], f32, kind="ExternalInput")
    tk_d = nc.dram_tensor("tk", [B, MAXLEN], f32, kind="ExternalInput")
    E_d = nc.dram_tensor("emb", [V, EMB], f32, kind="ExternalInput")
    krows = 257 if use_bias else 256
    wp_d = nc.dram_tensor("wp", [krows, 3 * SH], f32m, kind="ExternalInput")

    out_d = nc.dram_tensor("out_shard", [B, SH], f32, kind="ExternalOutput")
    tko_d = nc.dram_tensor("tok_out", [B, MAXLEN], f32, kind="ExternalOutput")
    unf_d = nc.dram_tensor("unf_out", [B, LAT], f32, kind="ExternalOutput")

    with tile.TileContext(nc) as tc:
        with (
            tc.tile_pool(name="cpool", bufs=1) as cpool,
            tc.tile_pool(name="wpool", bufs=1) as wpool,
            tc.tile_pool(name="ppool", bufs=1, space="PSUM") as ppool,
            tc.tile_pool(name="zpool", bufs=1, space="PSUM") as zpool,
            tc.tile_pool(name="dpool", bufs=1, space="DRAM") as dpool,
        ):
            # Prefetch the exp_and_others ACT table set (tanh+exp) while DMAs run.
            warm = cpool.tile([1, 8], f32, tag="warm")
            nc.vector.memset(warm[:], 0.0)
            nc.scalar.activation(warm[:], warm[:], AF.Tanh)

            # Warm the PE HAM clock gate: keep PE busy from t=0 so the real
            # matmuls run at full clock. Dummy bf16 matmuls on a zero tile.
            wz = cpool.tile([128, 256], mybir.dt.bfloat16, tag="wz")
            nc.vector.memset(wz[:], 0.0)
            wps = ppool.tile([128, 256], f32, tag="wps")
            for _ in range(24):
                nc.tensor.matmul(wps[:], wz[:, :128], wz[:],
                                 start=True, stop=True)

            # identity for the PE transposes (no deps; build early on Pool)
            ident = cpool.tile([B, B], f32, tag="ident")
            make_identity(nc, ident[:])

            # ---- decode input first: it heads the critical path ----
            ip_t = cpool.tile([B, LAT], f32, tag="ip")
            nc.sync.dma_start(ip_t[:], ip_d[:])

            # ---- decode: token = floor(p*4096), new_coord = frac(p*4096) ----
            xcol = cpool.tile([B, 1], f32, tag="xcol")
            nc.vector.tensor_scalar_mul(xcol[:], ip_t[:, cd:cd + 1], float(V))

            tokf = cpool.tile([B, 1], f32, tag="tokf")
            frac = cpool.tile([B, 1], f32, tag="frac")
            if token_mode == "mod":
                # floor(x) via int cast (any rounding mode) + fixup:
                #   r = round(x); r -= (r > x)
                toki_r = cpool.tile([B, 1], i32, tag="toki_r")
                nc.vector.tensor_copy(toki_r[:], xcol[:])
                tokr_f = cpool.tile([B, 1], f32, tag="tokr_f")
                nc.vector.tensor_copy(tokr_f[:], toki_r[:])
                too_big = cpool.tile([B, 1], f32, tag="too_big")
                nc.vector.tensor_scalar(out=too_big[:], in0=tokr_f[:],
                                        scalar1=xcol[:, :1], scalar2=None,
                                        op0=OP.is_gt)
                nc.vector.tensor_tensor(out=tokf[:], in0=tokr_f[:],
                                        in1=too_big[:], op=OP.subtract)
                nc.vector.tensor_tensor(out=frac[:], in0=xcol[:], in1=tokf[:],
                                        op=OP.subtract)
            else:
                # grid[j, n] = n+1 (same for every partition); count grid <= x
                grid_i = cpool.tile([B, V], i32, tag="grid_i")
                nc.gpsimd.iota(grid_i[:], pattern=[[1, V]], base=1,
                               channel_multiplier=0)
                grid_f = cpool.tile([B, V], f32, tag="grid_f")
                nc.vector.tensor_copy(grid_f[:], grid_i[:])
                cmp_t = cpool.tile([B, V], f32, tag="cmp")
                nc.vector.tensor_scalar(out=cmp_t[:], in0=grid_f[:],
                                        scalar1=xcol[:, :1], scalar2=None,
                                        op0=OP.is_le)
                nc.vector.reduce_sum(tokf[:], cmp_t[:],
                                     axis=mybir.AxisListType.X)
                nc.vector.tensor_tensor(out=frac[:], in0=xcol[:], in1=tokf[:],
                                        op=OP.subtract)

            toki = cpool.tile([B, 1], i32, tag="toki")
            nc.vector.tensor_copy(toki[:], tokf[:])
            maskp = cpool.tile([B, 1], f32, tag="maskp")
            nc.vector.tensor_scalar(out=maskp[:], in0=tokf[:], scalar1=0.5,
                                    scalar2=None, op0=OP.is_ge)

            # ---- embedding gather x = E[token]  [B, EMB] ----
            x_t = cpool.tile([B, EMB], f32, tag="x")
            nc.gpsimd.indirect_dma_start(
                out=x_t[:], out_offset=None, in_=E_d[:],
                in_offset=bass.IndirectOffsetOnAxis(ap=toki[:, :1], axis=0))

            # ---- transpose x -> xT chunks [128, B] ----
            xT = []
            for c in range(2):
                tp = ppool.tile([128, B], f32, tag=f"tp{c}")
                nc.tensor.transpose(out=tp[:], in_=x_t[:, c * 128:(c + 1) * 128],
                                    identity=ident[:])
                xt_sb = cpool.tile([128, B], f32m, tag=f"xT{c}")
                nc.vector.tensor_copy(xt_sb[:], tp[:])
                xT.append(xt_sb)

            # ---- weight shard load: 6 blocks of [128,512], in need-order,
            # alternating between the two HWDGE queues (SP / ACT) ----
            wblk = {}
            for gi in range(3):
                for c in range(2):
                    w = wpool.tile([128, SH], f32m, tag=f"w{gi}_{c}")
                    eng = nc.sync if c == 0 else nc.scalar
                    eng.dma_start(
                        w[:], wp_d[c * 128:(c + 1) * 128,
                                   gi * SH:(gi + 1) * SH])
                    wblk[(gi, c)] = w
            if use_bias:
                br = wpool.tile([1, 3 * SH], f32, tag="br")
                nc.sync.dma_start(br[:], wp_d[256:257, :])
                ones1 = cpool.tile([1, B], f32, tag="ones1")
                nc.vector.memset(ones1[:], 1.0)

            # tokens input load (off the critical path, behind the weights)
            tk_t = cpool.tile([B, MAXLEN], f32, tag="tk")
            nc.scalar.dma_start(tk_t[:], tk_d[:])

            # ---- gate matmuls + activations, pipelined in column halves ----
            # gate blocks in wp: 0 -> i, 1 -> g, 2 -> o
            # sigmoid(z) = 0.5*tanh(0.5 z) + 0.5 (tanh & exp share a table set)
            HS = SH // 2
            e_t = cpool.tile([B, SH], f32, tag="e_t")

            def gate(gi, hh, pre, tag):
                # hh None -> full width [B, SH]; else one [B, HS] half
                if hh is None:
                    hsl = slice(0, SH)
                    bsl = slice(gi * SH, (gi + 1) * SH)
                else:
                    hsl = slice(hh * HS, (hh + 1) * HS)
                    bsl = slice(gi * SH + hh * HS, gi * SH + (hh + 1) * HS)
                n = hsl.stop - hsl.start
                zg = zpool.tile([B, n], f32, tag=f"z{tag}")
                nc.tensor.matmul(zg[:], xT[0][:], wblk[(gi, 0)][:, hsl],
                                 start=True, stop=False)
                nc.tensor.matmul(zg[:], xT[1][:], wblk[(gi, 1)][:, hsl],
                                 start=False, stop=not use_bias)
                if use_bias:
                    nc.tensor.matmul(zg[:], ones1[:], br[:, bsl],
                                     start=False, stop=True)
                a = cpool.tile([B, n], f32, tag=f"a{tag}")
                nc.scalar.activation(a[:], zg[:], AF.Tanh, scale=pre)
                return a

            # ACT FIFO order matters: i/g tanhs first at full width (fewer
            # per-op overheads; not gated on DVE work), then the o-gate tanh
            # halves, then the tanh(c) halves, then the exp halves — so the
            # tail pipelines and the queue never head-of-line blocks on DVE.
            a_i = gate(0, None, 0.5, "i")
            a_g = gate(1, None, 1.0, "g")
            c_t = {}
            for hh in range(2):
                hsl = slice(hh * HS, (hh + 1) * HS)
                i_h = cpool.tile([B, HS], f32, tag=f"i{hh}")
                nc.vector.tensor_scalar(out=i_h[:], in0=a_i[:, hsl],
                                        scalar1=0.5, scalar2=0.5,
                                        op0=OP.mult, op1=OP.add)
                c_h = cpool.tile([B, HS], f32, tag=f"c{hh}")
                nc.vector.tensor_tensor(out=c_h[:], in0=i_h[:],
                                        in1=a_g[:, hsl], op=OP.mult)
                c_t[hh] = c_h

            o_t = {}
            for hh in range(2):
                a_o = gate(2, hh, 0.5, f"o{hh}")
                o_h = cpool.tile([B, HS], f32, tag=f"o{hh}")
                nc.vector.tensor_scalar(out=o_h[:], in0=a_o[:], scalar1=0.5,
                                        scalar2=0.5, op0=OP.mult, op1=OP.add)
                o_t[hh] = o_h

            tch = {}
            for hh in range(2):
                tch_h = cpool.tile([B, HS], f32, tag=f"tch{hh}")
                nc.scalar.activation(tch_h[:], c_t[hh][:], AF.Tanh)
                tch[hh] = tch_h

            # local row-sums: one [64,1] accum per half
            slocs = []
            for hh in range(2):
                hsl = slice(hh * HS, (hh + 1) * HS)
                h_t = cpool.tile([B, HS], f32, tag=f"h{hh}")
                nc.vector.tensor_tensor(out=h_t[:], in0=o_t[hh][:],
                                        in1=tch[hh][:], op=OP.mult)
                sl = cpool.tile([B, 1], f32, tag=f"sl{hh}")
                nc.scalar.activation(e_t[:, hsl], h_t[:], AF.Exp,
                                     scale=maskp[:, :1], accum_out=sl[:])
                slocs.append(sl)

            # ---- tokens / unfolding outputs (overlap with the collective) ----
            nc.vector.tensor_copy(tk_t[:, ts:ts + 1], tokf[:])
            nc.scalar.dma_start(tko_d[:], tk_t[:])
            un_t = cpool.tile([B, LAT], f32, tag="un")
            nc.vector.tensor_copy(un_t[:], ip_t[:])
            nc.vector.tensor_copy(un_t[:, cd:cd + 1], frac[:])
            nc.scalar.dma_start(unf_d[:], un_t[:])

            # ---- AllGather the 8 local [2*64,1] half-sum vectors ----
            # cin = [sums(half0); sums(half1)], each written by its own DMA
            # so the half-0 write overlaps exp(h1)
            parts = cpool.tile([NCORES, 2 * B], f32, tag="parts")
            if no_cc:
                # timing-analysis variant only: skip the collective
                nc.vector.memset(parts[:], float(HS))
            else:
                cin = dpool.tile([2 * B, 1], f32, tag="cin")
                nc.sync.dma_start(cin[0:B, :], slocs[0][:])
                nc.sync.dma_start(cin[B:2 * B, :], slocs[1][:])
                cout = dpool.tile([NCORES * 2 * B, 1], f32, tag="cout")
                nc.gpsimd.collective_compute(
                    "AllGather", mybir.AluOpType.bypass,
                    replica_groups=[list(range(NCORES))],
                    ins=[cin[:].opt()], outs=[cout[:].opt()])
                nc.sync.dma_start(
                    parts[:],
                    cout[:].rearrange("(k hb) o -> k (hb o)", k=NCORES))

            # S[b] = sum over cores and halves via two accumulating matmuls
            ones8 = cpool.tile([NCORES, 1], f32, tag="ones8")
            nc.vector.memset(ones8[:], 1.0)
            S_ps = ppool.tile([B, 1], f32, tag="S")
            pv = parts[:].rearrange("k (h b) -> k h b", h=2)
            nc.tensor.matmul(S_ps[:], pv[:, 0, :], ones8[:],
                             start=True, stop=False)
            nc.tensor.matmul(S_ps[:], pv[:, 1, :], ones8[:],
                             start=False, stop=True)
            s_sb = cpool.tile([B, 1], f32, tag="s_sb")
            nc.vector.reciprocal(s_sb[:], S_ps[:])

            # normalize + store in halves on both HWDGE queues: the half-0
            # store overlaps the half-1 multiply
            out_t = cpool.tile([B, SH], f32, tag="out_t")
            nc.vector.tensor_scalar_mul(out_t[:, 0:HS], e_t[:, 0:HS],
                                        s_sb[:, :1])
            nc.sync.dma_start(out_d[:, 0:HS], out_t[:, 0:HS])
            nc.vector.tensor_scalar_mul(out_t[:, HS:SH], e_t[:, HS:SH],
                                        s_sb[:, :1])
            nc.scalar.dma_start(out_d[:, HS:SH], out_t[:, HS:SH])

    nc.compile()
    return nc


def _get_program(cd: int, ts: int, use_bias: bool):
    key = (cd, ts, use_bias, TOKEN_MODE, F32R)
    if key not in _prog_cache:
        _prog_cache[key] = _build_program(cd, ts, use_bias, TOKEN_MODE,
                                          f32r=F32R)
    return _prog_cache[key]


def _get_pjrt_runner(cd: int, ts: int, use_bias: bool):
    """Compile the program once per key into a cached jitted shard_map call."""
    key = (cd, ts, use_bias, TOKEN_MODE, F32R)
    if key in _runner_cache:
        return _runner_cache[key]

    import jax
    from jax.sharding import Mesh, PartitionSpec
    try:
        from jax.experimental.shard_map import shard_map
    except ImportError:  # newer jax
        from jax.sharding import shard_map  # type: ignore

    from concourse import bass2jax, mybir
    from concourse.bass2jax import _bass_exec_p, partition_id_tensor

    nc = _get_program(cd, ts, use_bias)
    bass2jax.install_neuronx_cc_hook()
    partition_name = (nc.partition_id_tensor.name
                     if nc.partition_id_tensor else None)
    in_names, out_names, out_avals, zero_outs = [], [], [], []
    for alloc in nc.m.functions[0].allocations:
        if not isinstance(alloc, mybir.MemoryLocationSet):
            continue
        name = alloc.memorylocations[0].name
        if alloc.kind == "ExternalInput":
            if name != partition_name:
                in_names.append(name)
        elif alloc.kind == "ExternalOutput":
            out_names.append(name)
            shape = tuple(alloc.tensor_shape)
            dtype = mybir.dt.np(alloc.dtype)
            out_avals.append(jax.core.ShapedArray(shape, dtype))
            zero_outs.append(np.zeros(shape, dtype))
    n_params = len(in_names)
    all_in_names = list(in_names) + list(out_names)
    if partition_name is not None:
        all_in_names.append(partition_name)

    def _body(*args):
        operands = list(args)
        if partition_name is not None:
            operands.append(partition_id_tensor())
        outs = _bass_exec_p.bind(
            *operands,
            out_avals=tuple(out_avals),
            in_names=tuple(all_in_names),
            out_names=tuple(out_names),
            lowering_input_output_aliases=(),
            sim_require_finite=True,
            sim_require_nnan=True,
            nc=nc,
        )
        return tuple(outs)

    devices = jax.devices()[:NCORES]
    mesh = Mesh(np.asarray(devices), ("core",))
    in_specs = (PartitionSpec("core"),) * (n_params + len(out_names))
    out_specs = (PartitionSpec("core"),) * len(out_names)
    fn = jax.jit(
        shard_map(_body, mesh=mesh, in_specs=in_specs, out_specs=out_specs,
                  check_rep=False),
        keep_unused=True,
    )

    def run(in_maps):
        concat_in = [
            np.concatenate([np.asarray(in_maps[c][n]) for c in range(NCORES)],
                           axis=0) for n in in_names]
        concat_zeros = [np.zeros((NCORES * z.shape[0], *z.shape[1:]), z.dtype)
                        for z in zero_outs]
        outs = fn(*concat_in, *concat_zeros)
        return [
            {name: np.asarray(outs[i]).reshape(NCORES, *out_avals[i].shape)[c]
             for i, name in enumerate(out_names)}
            for c in range(NCORES)
        ]

    _runner_cache[key] = run
    return run


# ----------------------------------------------------------------------------
# Host fallback (mirrors reference.py in numpy; not used for the harness
# inputs, kept for general correctness)
# ----------------------------------------------------------------------------
def _softmax_np(x):
    m = np.max(x, axis=-1, keepdims=True)
    e = np.exp((x - m).astype(np.float32)).astype(np.float32)
    return (e / np.sum(e, axis=-1, keepdims=True)).astype(np.float32)


def _sigmoid_np(x):
    return (1.0 / (1.0 + np.exp(-x.astype(np.float32)))).astype(np.float32)


def _lstm_lm_np(token_ids, E, Wi, Wh, b):
    x = E[token_ids]                       # [B, T, EMB]
    maskv = token_ids != PAD
    Bt, T = token_ids.shape
    Vu = Wh.shape[0]
    h = np.zeros((Bt, Vu), np.float32)
    c = np.zeros((Bt, Vu), np.float32)
    for t in range(T):
        z = (x[:, t] @ Wi + h @ Wh + b).astype(np.float32)
        i = _sigmoid_np(z[:, :Vu])
        f = _sigmoid_np(z[:, Vu:2 * Vu])
        g = np.tanh(z[:, 2 * Vu:3 * Vu]).astype(np.float32)
        o = _sigmoid_np(z[:, 3 * Vu:])
        c_new = (f * c + i * g).astype(np.float32)
        h_new = (o * np.tanh(c_new)).astype(np.float32)
        m = maskv[:, t][:, None]
        h = np.where(m, h_new, h)
        c = np.where(m, c_new, c)
    return _softmax_np(h)


def _reference_np(input_point, one_softmax, tokens, unfolding_point, E, Wi, Wh,
                  b, curDim, timeStep):
    lat_dim = unfolding_point.shape[-1]
    pad_seq = np.full((input_point.shape[0], 1), PAD, np.int32)
    initial_softmax = _lstm_lm_np(pad_seq, E, Wi, Wh, b)
    if timeStep > 0:
        unfolding_point = input_point
        one_softmax = initial_softmax
    cums = np.cumsum(one_softmax, axis=1, dtype=np.float32)
    cums_excl = (cums - one_softmax).astype(np.float32)
    point = unfolding_point[:, curDim][:, None]
    inside = (cums > point) & (cums_excl <= point)
    token = np.argmax(inside, axis=1)
    low = np.take_along_axis(cums_excl, token[:, None], axis=1)
    size = np.take_along_axis(one_softmax, token[:, None], axis=1)
    new_coord = ((point - low) / size).astype(np.float32)
    unfolding_point = unfolding_point.copy()
    unfolding_point[:, curDim] = new_coord[:, 0]
    tokens = tokens.copy()
    tokens[:, timeStep] = token.astype(tokens.dtype)
    tokens_in = tokens[:, :timeStep + 1].astype(np.int32)
    one_softmax = _lstm_lm_np(tokens_in, E, Wi, Wh, b)
    curDim_new = 0 if curDim + 1 >= lat_dim else curDim + 1
    return (tokens, one_softmax, unfolding_point, np.float32(curDim_new),
            np.int32(timeStep + 1))


# ----------------------------------------------------------------------------
# Entry point
# ----------------------------------------------------------------------------
def kernel(input_point, one_softmax, tokens, unfolding_point, E, Wi, Wh, b,
           curDim, timeStep):
    global LAST_RESULTS
    f32 = np.float32
    input_point = np.ascontiguousarray(np.asarray(input_point, f32))
    one_softmax = np.ascontiguousarray(np.asarray(one_softmax, f32))
    tokens = np.ascontiguousarray(np.asarray(tokens, f32))
    unfolding_point = np.ascontiguousarray(np.asarray(unfolding_point, f32))
    E = np.ascontiguousarray(np.asarray(E, f32))
    Wi = np.ascontiguousarray(np.asarray(Wi, f32))
    Wh = np.asarray(Wh, f32)
    b = np.ascontiguousarray(np.asarray(b, f32))
    cd = int(np.asarray(curDim))
    ts = int(np.asarray(timeStep))

    p = input_point[:, cd] if 0 <= cd < input_point.shape[1] else None
    fast = (
        0 < ts < tokens.shape[1]
        and 0 <= cd < input_point.shape[1]
        and tokens.shape == (B, MAXLEN)
        and input_point.shape == (B, LAT)
        and E.shape == (V, EMB)
        and Wi.shape == (EMB, 4 * V)
        and bool(np.all(tokens[:, :ts] == 0.0))
        and bool(np.all((p >= 0.0) & (p < 1.0)))
    )
    if not fast:
        return _reference_np(input_point, one_softmax, tokens, unfolding_point,
                             E, Wi, Wh, b, cd, ts)

    use_bias = bool(np.any(b != 0.0))
    nc = _get_program(cd, ts, use_bias)

    # per-core packed weights: i/g/o gate columns of this core's hidden slice
    in_maps = []
    for k in range(NCORES):
        lo, hi = k * SH, (k + 1) * SH
        blocks = [Wi[:, lo:hi], Wi[:, 2 * V + lo:2 * V + hi],
                  Wi[:, 3 * V + lo:3 * V + hi]]
        wp = np.concatenate(blocks, axis=1)
        if use_bias:
            brow = np.concatenate([b[lo:hi], b[2 * V + lo:2 * V + hi],
                                   b[3 * V + lo:3 * V + hi]])[None, :]
            wp = np.concatenate([wp, brow], axis=0)
        in_maps.append({
            "ip": input_point,
            "tk": tokens,
            "emb": E,
            "wp": np.ascontiguousarray(wp, f32),
        })

    try:
        from concourse._compat import axon_active
        if axon_active() and not TRACE:
            # cached jitted executable: no retrace/recompile on repeat calls
            outs = _get_pjrt_runner(cd, ts, use_bias)(in_maps)
        else:
            from concourse import bass_utils
            res = bass_utils.run_bass_kernel_spmd(
                nc, in_maps, core_ids=list(range(NCORES)), trace=TRACE)
            LAST_RESULTS = res
            outs = res.results
    except Exception as e:  # no devices / runtime unavailable: stay correct
        import sys
        print(f"kernel.py: device path failed ({type(e).__name__}: {e}); "
              f"falling back to host reference", file=sys.stderr)
        return _reference_np(input_point, one_softmax, tokens,
                             unfolding_point, E, Wi, Wh, b, cd, ts)

    one_softmax_out = np.concatenate(
        [outs[k]["out_shard"] for k in range(NCORES)], axis=1)
    tokens_out = outs[0]["tok_out"]
    unf_out = outs[0]["unf_out"]

    curDim_new = 0 if cd + 1 >= LAT else cd + 1
    return (tokens_out, one_softmax_out, unf_out, np.float32(curDim_new),
            np.int32(ts + 1))
